# revision 50
# baseline (speedup 1.0000x reference)
"""Fused varlen SigLIP attention block for TRN2, tensor-parallel over heads
across 8 NeuronCores (2 heads per core).

Final schedule (~215us HW, vs 315us staged baseline):
  - Host packs all inputs per-partition-contiguous (one large descriptor
    per partition); x is chunked to the plan so the first qkv matmul
    starts ~5us in.
  - Phase 1 per t-tile: qkv psum[tl,432] (9 matmuls), rope entirely on DVE
    (0-stride broadcast cos/sin views, bf16 intermediates for the 4x mode;
    GpSimd would thrash its ucode library between tensor_tensor and
    phase-2 partition_broadcast — a ~7us reload), 4 PE transposes ->
    QK[72,4,T] (copy-out on DVE), v evacuated by ACT (idle in phase 1)
    into vseg with a ones column so PV emits the softmax row-sum free.
    Phase 1 measures 100% PE occupancy.
  - Phase 2 per chunk-pair/head, smallest pair first then largest-first:
    scores into a [128,1024] 2-bank psum so exp runs as one wide ACT
    instr; PV deferred one s-tile behind scores; the previous pair's
    out-proj block between pairs gives ACT catch-up headroom.
  - Normalize: rowsum copy (ACT) + raw ctx evac bf16 (DVE) free the cx
    bank early; reciprocal (DVE) + partition broadcast (GpSimd, library
    pre-warmed at kernel start) + multiply (DVE) run off-bank.
  - Out-proj evac lands in a per-chunk [128,9,512] bf16 staging tile, one
    DMA per chunk; evac alternates DVE/ACT in the kernel tail where ACT
    has no exp backlog. Host sums the 8 partials in f64.
  - PSUM banks: phase1 qkv(3)+tp(1); phase2 sc(2x2)+cx(2)+ou(2), with the
    tp shrink leaving fresh banks so the first scores don't wait on
    phase-1 psum drain.

bqkv/bout are zeros per spec; bout is still added on the host.
"""
import numpy as np
from contextlib import ExitStack

import ml_dtypes
import concourse.bass as bass
import concourse.bacc as bacc
import concourse.tile as tile
import concourse.mybir as mybir
from concourse import bass_utils

F32 = mybir.dt.float32
BF16 = mybir.dt.bfloat16

H = 1152
NH = 16
HD = 72
HD2 = 36
T = 4096
NCORES = 8
HPC = NH // NCORES          # heads per core
OC = 3 * HPC * HD           # 432
SCALE = HD ** -0.5
EXP_BIAS = -4.0

_CACHE = {}


def _plan(cu):
    bs = sorted(set([0, T] + [int(v) for v in cu[1:] if 0 < int(v) < T]))
    segs = [(a, b) for a, b in zip(bs[:-1], bs[1:]) if b > a]
    plan = []
    for (a, b) in segs:
        chunks = []
        c0 = a
        while c0 < b:
            cn = min(512, b - c0)
            tls = []
            t0 = c0
            while t0 < c0 + cn:
                tl = min(128, c0 + cn - t0)
                tls.append((t0, tl))
                t0 += tl
            chunks.append((c0, cn, tuple(tls)))
            c0 += cn
        plan.append((a, b, tuple(chunks)))
    return tuple(plan)


def _all_tiles(plan):
    out = []
    for a, b, chunks in plan:
        for c0, cn, tls in chunks:
            out.extend(tls)
    return out


def _stiles(a, b):
    sts = []
    s0 = a
    while s0 < b:
        sn = min(128, b - s0)
        sts.append((s0, sn))
        s0 += sn
    return sts


def _bcast(ap2d, n):
    """[p, d] view -> [p, n, d] with 0-stride middle dim."""
    return bass.AP(ap2d.tensor, ap2d.offset, [ap2d.ap[0], [0, n], ap2d.ap[1]])


def build(nc, plan):
    tiles = _all_tiles(plan)
    nt = len(tiles)
    tidx = {t0: i for i, (t0, tl) in enumerate(tiles)}
    chunks_all = [(c0, cn) for a, b, chs in plan for (c0, cn, tls) in chs]
    ncks = len(chunks_all)
    cidx = {c0: ci for ci, (c0, cn) in enumerate(chunks_all)}

    xP = nc.dram_tensor("xP", [128, ncks, 9, 512], BF16,
                        kind="ExternalInput").ap()
    wqP = nc.dram_tensor("wqP", [128, 9, OC], BF16, kind="ExternalInput").ap()
    woP = nc.dram_tensor("woP", [HD, HPC, H], BF16, kind="ExternalInput").ap()
    csP = nc.dram_tensor("csP", [128, nt, 2 * HD2], BF16, kind="ExternalInput").ap()
    idd = nc.dram_tensor("idd", [128, 128], BF16, kind="ExternalInput").ap()
    out4 = nc.dram_tensor("out4", [ncks, 128, 9, 512], BF16,
                          kind="ExternalOutput").ap()

    with tile.TileContext(nc) as tc, ExitStack() as ctx:
        P = lambda **kw: ctx.enter_context(tc.tile_pool(**kw))
        singles = P(name="singles", bufs=1)

        wq_sb = singles.tile([128, 9, OC], BF16)
        wo_sb = singles.tile([HD, HPC, H], BF16)
        cssb = singles.tile([128, nt, 2 * HD2], BF16)
        ident = singles.tile([128, 128], BF16)
        ebias = singles.tile([128, 1], F32)
        QK = singles.tile([HD, 4, T], BF16)
        vseg = singles.tile([128, nt, 194], BF16)   # per head: v(72) z(24) one(1)

        # wq first halves + first x chunk (issued inside phase 1, before
        # cs/ident/wo) are all the first qkv matmul needs to start
        nc.sync.dma_start(out=wq_sb[:, 0:5, :], in_=wqP[:, 0:5, :])
        nc.vector.memset(ebias, EXP_BIAS)
        # ones/zeros zones of vseg, set once for all tiles
        nc.gpsimd.memset(vseg[:, :, 72:96], 0.0)
        nc.gpsimd.memset(vseg[:, :, 96:97], 1.0)
        nc.gpsimd.memset(vseg[:, :, 169:193], 0.0)
        nc.gpsimd.memset(vseg[:, :, 193:194], 1.0)
        # GpSimd's only other op is phase-2 partition_broadcast; issue a
        # dummy one now so its ~7us ucode library reload hides under the
        # initial DMA loads instead of stalling the first normalize
        warm = singles.tile([HD, 1], F32)
        nc.gpsimd.partition_broadcast(warm, ebias[0:1, 0:1])

        # ---------------- phase 1: qkv + rope + transpose ----------------
        with ExitStack() as p1:
            P1 = lambda **kw: p1.enter_context(tc.tile_pool(**kw))
            # qkv(3)+tp(1) leaves 4 fresh banks for phase 2's score pool, so
            # the first scores matmul doesn't wait on phase-1 psum drain
            ps_qkv = P1(name="ps_qkv", bufs=3, space="PSUM")
            ps_tp = P1(name="ps_tp", bufs=1, space="PSUM")
            tmpd = P1(name="tmpd", bufs=3)
            tmpg = P1(name="tmpg", bufs=3)
            stp = P1(name="stp", bufs=3)
            xin = P1(name="xin", bufs=3)

            xts = {}

            def load_chunk(c0, cn, tls):
                ci = cidx[c0]
                xt = xin.tile([128, 9, 512], BF16, tag="xt", name=f"xt_{c0}")
                nc.sync.dma_start(out=xt[:, :, :cn], in_=xP[:, ci, :, :cn])
                for (t0, tl) in tls:
                    xts[t0] = (xt, t0 - c0)

            def qkv_mm(t0, tl):
                i = tidx[t0]
                ps = ps_qkv.tile([128, OC], F32, tag="psq", name=f"psq_{i}")
                xt, off = xts[t0]
                for kt in range(9):
                    nc.tensor.matmul(ps[:tl, :], xt[:, kt, off:off + tl],
                                     wq_sb[:, kt, :], start=(kt == 0),
                                     stop=(kt == 8))
                return ps

            def rope_dve(t0, tl, ps):
                i = tidx[t0]
                qk = ps[:tl, 0:288].rearrange("p (j h d) -> p j h d", h=2, d=36)
                px1 = qk[:, :, 0, :]
                px2 = qk[:, :, 1, :]
                c = _bcast(cssb[:tl, i, 0:36], 4)
                s = _bcast(cssb[:tl, i, 36:72], 4)
                m1 = tmpd.tile([128, 4, 36], BF16, tag="m1", name=f"m1_{i}")
                m2 = tmpd.tile([128, 4, 36], BF16, tag="m2", name=f"m2_{i}")
                m3 = tmpg.tile([128, 4, 36], BF16, tag="m3", name=f"m3_{i}")
                m4 = tmpg.tile([128, 4, 36], BF16, tag="m4", name=f"m4_{i}")
                stg = stp.tile([128, 4, 2, 36], BF16, tag="stg", name=f"stg_{i}")
                # all on DVE (GpSimd would thrash its ucode library between
                # tensor_tensor and phase-2 partition_broadcast); the bf16
                # SBUF-only sub/add run in the DVE 4x mode
                nc.vector.tensor_tensor(out=m1[:tl], in0=px1, in1=c,
                                        op=mybir.AluOpType.mult)
                nc.vector.tensor_tensor(out=m2[:tl], in0=px2, in1=s,
                                        op=mybir.AluOpType.mult)
                nc.vector.tensor_tensor(out=m3[:tl], in0=px2, in1=c,
                                        op=mybir.AluOpType.mult)
                nc.vector.tensor_tensor(out=m4[:tl], in0=px1, in1=s,
                                        op=mybir.AluOpType.mult)
                nc.vector.tensor_tensor(out=stg[:tl, :, 0, :], in0=m1[:tl],
                                        in1=m2[:tl], op=mybir.AluOpType.subtract)
                nc.vector.tensor_tensor(out=stg[:tl, :, 1, :], in0=m3[:tl],
                                        in1=m4[:tl], op=mybir.AluOpType.add)
                return stg

            def tp_part(t0, tl, ps, stg):
                i = tidx[t0]
                pt = ps_tp.tile([HD, 512], BF16, tag="pt", name=f"pt_{i}")
                stgf = stg.rearrange("p j h d -> p (j h d)")
                for j in range(4):
                    nc.tensor.transpose(pt[:, j * tl:(j + 1) * tl],
                                        stgf[:tl, j * 72:(j + 1) * 72],
                                        ident[:tl, :tl])
                nc.vector.tensor_copy(QK[:, :, t0:t0 + tl],
                                      pt[:, 0:4 * tl].rearrange(
                                          "d (j t) -> d j t", j=4))
                # v evacuation on ACT (idle during phase 1)
                nc.scalar.copy(vseg[:tl, i, 0:72], ps[:tl, 288:360])
                nc.scalar.copy(vseg[:tl, i, 97:169], ps[:tl, 360:432])

            def rope_tp(t0, tl, ps):
                tp_part(t0, tl, ps, rope_dve(t0, tl, ps))

            # first x chunk right behind wq, then the other singles loads
            # (cs for rope, ident for transposes, wo only for outproj)
            first_chunk = plan[0][2][0]
            load_chunk(first_chunk[0], first_chunk[1], first_chunk[2])
            nc.sync.dma_start(out=wq_sb[:, 5:9, :], in_=wqP[:, 5:9, :])
            nc.sync.dma_start(out=cssb, in_=csP)
            nc.sync.dma_start(out=ident, in_=idd)
            nc.sync.dma_start(out=wo_sb, in_=woP)

            pending = None
            for a, b, chunks in plan:
                for c0, cn, tls in chunks:
                    if c0 != first_chunk[0]:
                        load_chunk(c0, cn, tls)
                    for (t0, tl) in tls:
                        ps = qkv_mm(t0, tl)
                        if pending is not None:
                            rope_tp(*pending)
                        pending = (t0, tl, ps)
            if pending is not None:
                rope_tp(*pending)

        # ---------------- phase 2: attention + out-proj ------------------
        with ExitStack() as p2:
            P2 = lambda **kw: p2.enter_context(tc.tile_pool(**kw))
            ps_sc = P2(name="ps_sc", bufs=2, space="PSUM")   # 2 banks each
            ps_cx = P2(name="ps_cx", bufs=2, space="PSUM")
            ps_ou = P2(name="ps_ou", bufs=2, space="PSUM")
            esp = P2(name="esp", bufs=6)
            bcp = P2(name="bcp", bufs=3)
            crp = P2(name="crp", bufs=3)
            cxp = P2(name="cxp", bufs=8)
            osb = P2(name="osb", bufs=3)

            def outproj_steps(pair, cxs01, act_evac=False):
                """Generator: the pair's out-proj emitted one m-step per
                next(). act_evac alternates evacuation DVE/ACT — only safe
                when ACT has no exp backlog (the kernel tail)."""
                for ci, (c0, cn) in enumerate(pair):
                    ck = cidx[c0]
                    ob = osb.tile([128, 9, 512], BF16, tag="ob",
                                  name=f"ob_{c0}")
                    for m in range(9):
                        po = ps_ou.tile([128, 512], F32, tag="po",
                                        name=f"po_{c0}_{m}")
                        for h in range(HPC):
                            nc.tensor.matmul(
                                po[:, :cn],
                                wo_sb[:, h, m * 128:(m + 1) * 128],
                                cxs01[h][ci][:, :cn],
                                start=(h == 0), stop=(h == HPC - 1))
                        if act_evac and m % 2 == 1:
                            nc.scalar.copy(ob[:, m, :cn], po[:, :cn])
                        else:
                            nc.vector.tensor_copy(ob[:, m, :cn], po[:, :cn])
                        yield
                    nc.sync.dma_start(out=out4[ck], in_=ob)

            def attn_pair(a, b, pair, h, filler, fill_every):
                sts = _stiles(a, b)
                totw = sum(cn for (c0, cn) in pair)
                cxs_out = []
                cx = {}
                for (c0, cn) in pair:
                    cx[c0] = ps_cx.tile([97, 512], F32, tag="cx",
                                        name=f"cx_{c0}_{h}")

                def pv_do(st, first, last):
                    s0, sn, es = st
                    i = tidx[s0]
                    off = 0
                    for (c0, cn) in pair:
                        nc.tensor.matmul(cx[c0][:, :cn],
                                         vseg[:sn, i, h * 97:(h + 1) * 97],
                                         es[:sn, off:off + cn],
                                         start=first, stop=last)
                        off += cn

                fill_acc = 0.0
                prev = None
                for si, (s0, sn) in enumerate(sts):
                    sc = ps_sc.tile([128, 1024], F32, tag="sc",
                                    name=f"sc_{pair[0][0]}_{h}_{si}")
                    off = 0
                    for (c0, cn) in pair:
                        nc.tensor.matmul(sc[:sn, off:off + cn],
                                         QK[:, 2 + h, s0:s0 + sn],
                                         QK[:, h, c0:c0 + cn],
                                         start=True, stop=True)
                        off += cn
                    es = esp.tile([128, 1024], BF16, tag="es",
                                  name=f"es_{pair[0][0]}_{h}_{si}")
                    nc.scalar.activation(es[:sn, :totw], sc[:sn, :totw],
                                         mybir.ActivationFunctionType.Exp,
                                         bias=ebias[:sn], scale=SCALE)
                    if prev is not None:
                        pv_do(prev, first=(si == 1), last=False)
                    prev = (s0, sn, es)
                    if fill_every > 0:
                        fill_acc += fill_every
                        while filler is not None and fill_acc >= 1.0:
                            fill_acc -= 1.0
                            if next(filler, "done") == "done":
                                filler = None
                pv_do(prev, first=(len(sts) == 1), last=True)

                # two passes: first free the cx banks (rowsum to SBUF on ACT,
                # raw ctx evac on DVE), then the broadcast/multiply chain —
                # so a slow chain for chunk A never delays chunk B's evac
                stage = []
                for (c0, cn) in pair:
                    rs = bcp.tile([1, 512], F32, tag="rs", name=f"rs_{c0}_{h}")
                    nc.scalar.copy(rs[:, :cn], cx[c0][96:97, :cn])
                    craw = crp.tile([HD, 512], BF16, tag="craw",
                                    name=f"craw_{c0}_{h}")
                    nc.vector.tensor_copy(craw[:, :cn], cx[c0][0:HD, :cn])
                    rr = bcp.tile([1, 512], F32, tag="rr", name=f"rr_{c0}_{h}")
                    nc.vector.reciprocal_approx_fast(out=rr[:, :cn],
                                                     in_=rs[:, :cn])
                    stage.append((c0, cn, craw, rr))
                for (c0, cn, craw, rr) in stage:
                    bc = bcp.tile([HD, 512], F32, tag="bc", name=f"bc_{c0}_{h}")
                    nc.gpsimd.partition_broadcast(bc[:, :cn], rr[:, :cn])
                    cxs = cxp.tile([HD, 512], BF16, tag="cxs",
                                   name=f"cxs_{c0}_{h}")
                    nc.vector.tensor_tensor(out=cxs[:, :cn], in0=craw[:, :cn],
                                            in1=bc[:, :cn],
                                            op=mybir.AluOpType.mult)
                    cxs_out.append(cxs)
                return cxs_out, filler

            work = []
            for a, b, chunks in plan:
                cl = [(c0, cn) for (c0, cn, tls) in chunks]
                for i in range(0, len(cl), 2):
                    work.append((a, b, tuple(cl[i:i + 2])))
            # First window has no deferred-outproj PE filler, so it must be
            # the pair with the smallest exp-vs-PE overrun (the smallest one);
            # the rest run largest-first so a small pair forms the tail.
            work.sort(key=lambda w: -(w[1] - w[0]) * sum(c[1] for c in w[2]))
            if len(work) > 1:
                work.insert(0, work.pop())

            filler = None
            for wi, (a, b, pair) in enumerate(work):
                cxs01 = []
                for h in range(HPC):
                    cxs, filler = attn_pair(a, b, pair, h, filler, 0)
                    cxs01.append(cxs)
                    # half the previous pair's outproj between the heads so
                    # ACT catches up on exp mid-pair, the rest after
                    if h == 0 and filler is not None:
                        for _ in range(5 * len(pair)):
                            if next(filler, "done") == "done":
                                filler = None
                                break
                if filler is not None:
                    for _ in filler:
                        pass
                filler = outproj_steps(pair, cxs01,
                                       act_evac=(wi >= len(work) - 2))
            if filler is not None:
                for _ in filler:
                    pass
    return nc


def _build_inputs(x, wqkv, wout, cos, sin, plan):
    tiles = _all_tiles(plan)
    nt = len(tiles)
    bf = ml_dtypes.bfloat16

    chunks_all = [(c0, cn) for a, b, chs in plan for (c0, cn, tls) in chs]
    xr = x.reshape(T, 9, 128).transpose(2, 1, 0)          # [128, 9, T]
    xP = np.zeros((128, len(chunks_all), 9, 512), np.float32)
    for ci, (c0, cn) in enumerate(chunks_all):
        xP[:, ci, :, :cn] = xr[:, :, c0:c0 + cn]
    xP = np.ascontiguousarray(xP).astype(bf)

    c = cos[:, :HD2]
    s = sin[:, :HD2]
    csP = np.zeros((128, nt, 2 * HD2), np.float32)
    for i, (t0, tl) in enumerate(tiles):
        csP[:tl, i, 0:HD2] = c[t0:t0 + tl]
        csP[:tl, i, HD2:2 * HD2] = s[t0:t0 + tl]
    csP = csP.astype(bf)
    idd = np.eye(128, dtype=np.float32).astype(bf)

    in_maps = []
    for core in range(NCORES):
        h0 = core * HPC
        rows = []
        for kind in range(3):
            for h in range(HPC):
                base = kind * H + (h0 + h) * HD
                rows.extend(range(base, base + HD))
        wq = np.ascontiguousarray(wqkv[rows, :].T)                 # [H, 432]
        wqP = np.ascontiguousarray(
            wq.reshape(9, 128, OC).transpose(1, 0, 2)).astype(bf)  # [128,9,432]
        cols = np.arange(h0 * HD, (h0 + HPC) * HD)
        wo = np.ascontiguousarray(wout[:, cols].T)                 # [144, H]
        woP = np.ascontiguousarray(
            wo.reshape(HPC, HD, H).transpose(1, 0, 2)).astype(bf)  # [72,2,H]
        in_maps.append({"xP": xP, "wqP": wqP, "woP": woP,
                        "csP": csP, "idd": idd})
    return in_maps


def kernel(hidden_states, wqkv, bqkv, wout, bout, cos, sin, cu_seqlens,
           _trace=False):
    x = np.asarray(hidden_states, np.float32).reshape(T, H)
    plan = _plan(np.asarray(cu_seqlens).astype(np.int64))
    if plan not in _CACHE:
        nc = bacc.Bacc("TRN2", target_bir_lowering=False, debug=False)
        build(nc, plan)
        nc.compile()
        _CACHE[plan] = nc
    nc = _CACHE[plan]
    in_maps = _build_inputs(x, np.asarray(wqkv, np.float32),
                            np.asarray(wout, np.float32),
                            np.asarray(cos, np.float32),
                            np.asarray(sin, np.float32), plan)
    res = bass_utils.run_bass_kernel_spmd(nc, in_maps,
                                          core_ids=list(range(NCORES)),
                                          trace=_trace)
    chunks_all = [(c0, cn) for a, b, chs in plan for (c0, cn, tls) in chs]
    out = np.zeros((H, T), np.float64)
    for core in range(NCORES):
        o4 = res.results[core]["out4"].astype(np.float64)
        for ci, (c0, cn) in enumerate(chunks_all):
            out[:, c0:c0 + cn] += o4[ci].transpose(1, 0, 2).reshape(
                H, 512)[:, :cn]
    out = out.T + np.asarray(bout, np.float64)[None, :]
    if _trace:
        kernel.last_exec_time_ns = res.exec_time_ns
        kernel.last_trace = res.instructions_and_trace
    return out.astype(np.float32).reshape(1, T, H)
